# revision 5
# baseline (speedup 1.0000x reference)
"""Multi-head cross-attention Trainium2 kernel (8 NeuronCores, SPMD).

Problem: nn_MultiHeadCrossAttention_31791347925263
  x:[4,2048,768], y:[4,2048,768], 12 heads x 64, fp32.
  out = softmax((x Wq^T)(y Wk^T)^T / 8 + mask) (y Wv^T) Wo^T   (+ zero biases)

Sharding: 8 cores = (batch b in 0..3) x (head-half g in 0..1). Each core
computes 6 heads over ALL 2048 queries x 2048 keys of its batch, plus its
partial output projection (Wo rows for its heads); the host sums the two
partials per batch during unshard (the "all-reduce after the output
projection" of the tensor-parallel head split, done for free off-device).

Design (v12). v11 measured: Tensor busy 248.9us (88%), ACT busy 212us
(75%), span 282.6us; HAM cold tail from 258us. Changes vs v11:
  - QK head-PAIRING via row-tiled concurrent matmuls: the two heads of a
    pair occupy partition rows 0-63 / 64-127 of the packed qT/kTp tiles;
    QK is 2 concurrent K=64 matmuls (tile_position (0,0)/(64,0)) instead
    of one zero-padded K=128 matmul. Per the PE tiling model (4-way K=32
    row tiling measures 3.07x) the pair streams in ~512 cycles, halving
    QK's 82us to ~41us. Also kills the kTz zero halves (saves 12KB SBUF
    + the memsets; kt chunk copies become one [128,512] cast).
  - Wider ACTIVATEs: scores go to a [128,2048] 4-bank slot (2 bursts)
    alternating with a [128,1024] 2-bank slot (1 burst); exp is 64 x
    (2000+1147)ns = 201us vs v11's 192 x 1147ns = 220us (the 352-cycle
    per-instruction overhead was 26% at free dim 1024; a PSUM bank is
    512 f32 so 2048 is the widest slot that leaves room for PV).
  - PSUM: S0 scores 4 banks + S1 scores 2 banks + vtA/vtB PV
    accumulators [65,512] 1 bank each = 8. Projection chunks share the
    S1 slot (tag rotation serializes them against the score stream).
  - 512-query granularity: bursts are (head-pair, query-quarter,
    key-block); PV accumulates [65,512]; the output projection runs
    per quarter as soon as its 3 head-pairs finish, so only quarter 3's
    4 o-chunks remain in the tail (v11 drained 8 chunks at HAM-cold
    clock from 258us).
  - v' (PV stationary) chunks split per head-pair (16x6 -> 48 chunks of
    6 matmuls x 128 cols) so unit 1 only forces hb0's 12K cycles, not
    the whole 37K.
  - DMA order: wk, y half1, wq, x half1, wv, y half2, wo, x half2.
    First ACTIVATE ~14us (v11: ~26us).
Kept from v11: all-bf16 (fp8 costs 2-3% err; no per-col PE win), the
65th ones-column PV denominator trick, demand-split x/y halves on one
strictly-ordered DMA queue, reciprocal_approx_fast + gpsimd broadcast
normalize, host-side all-reduce.
"""

import numpy as np

B, S, D = 4, 2048, 768
H, Dh = 12, 64
HC = H // 2          # 6 heads per core
DC = HC * Dh         # 384 projected dims per core
N_CORES = 8
DB = D // 128        # 6 d_model contraction blocks
OB = DC // 128       # 3 head-pair blocks (q/k/o contraction blocks)
SKB = S // 128       # 16 key blocks
NQ = 4               # query quarters
SQ4 = S // NQ        # 512 queries per quarter
VPW = HC * (Dh + 1)  # 390: v' width (64 v cols + 1 ones col per head)

PAIRED = True        # row-tiled concurrent K=64 QK matmuls

_cache = {}


def _build_nc():
    import concourse.mybir as mybir
    import concourse.tile as tile
    from concourse import bacc

    f32 = mybir.dt.float32
    bf16 = mybir.dt.bfloat16
    EXP = mybir.ActivationFunctionType.Exp

    nc = bacc.Bacc("TRN2", target_bir_lowering=False)
    x16 = nc.dram_tensor("x16", [128, DB, S], bf16, kind="ExternalInput")
    y16 = nc.dram_tensor("y16", [128, DB, S], bf16, kind="ExternalInput")
    wq16 = nc.dram_tensor("wq16", [128, DB, DC], bf16, kind="ExternalInput")
    wk16 = nc.dram_tensor("wk16", [128, DB, DC], bf16, kind="ExternalInput")
    wv16 = nc.dram_tensor("wv16", [128, DB, DC], bf16, kind="ExternalInput")
    wo16 = nc.dram_tensor("wo16", [128, OB, D], bf16, kind="ExternalInput")
    out = nc.dram_tensor("out", [S, D], f32, kind="ExternalOutput")

    with tile.TileContext(nc) as tc:
        with tc.tile_pool(name="persist", bufs=1) as pp, \
             tc.tile_pool(name="ps0", bufs=1, space="PSUM") as ps0, \
             tc.tile_pool(name="ps1", bufs=1, space="PSUM") as ps1, \
             tc.tile_pool(name="vta", bufs=1, space="PSUM") as vta_pool, \
             tc.tile_pool(name="vtb", bufs=1, space="PSUM") as vtb_pool, \
             tc.tile_pool(name="ptp", bufs=3) as pt_pool, \
             tc.tile_pool(name="nrm", bufs=2) as nrm_pool, \
             tc.tile_pool(name="osb", bufs=3) as o_pool:

            x16t = pp.tile([128, DB, S], bf16, name="x16t")
            y16t = pp.tile([128, DB, S], bf16, name="y16t")
            wq16t = pp.tile([128, DB, DC], bf16, name="wq16t")
            wk16t = pp.tile([128, DB, DC], bf16, name="wk16t")
            wv16t = pp.tile([128, DB, DC], bf16, name="wv16t")
            wo16t = pp.tile([128, OB, D], bf16, name="wo16t")

            # kTp[hb]: packed k^T for the pair: even head rows 0-63, odd
            # head rows 64-127 (same rows its q occupies in qT[hb])
            kTp = [pp.tile([128, S], bf16, name=f"kTp{i}") for i in range(OB)]
            qT = [pp.tile([128, S], bf16, name=f"qT{i}") for i in range(OB)]
            # vnorm[g2][ob]: normalized values for query quarter g2
            vnorm = [[pp.tile([128, SQ4], bf16, name=f"vn{g}_{i}")
                      for i in range(OB)] for g in range(NQ)]
            vp16 = [pp.tile([128, VPW], bf16, name=f"vp16_{i}")
                    for i in range(SKB)]
            vp3 = [t.rearrange("p (h c) -> p h c", c=Dh + 1) for t in vp16]

            # ---- input DMA, priority order (one strictly-ordered queue).
            # wk+y1 -> kt0 chunks; wq+x1 -> qt quarters 0-1; wv -> vp;
            # y2 -> kt c2/c3 (needed ~burst s=8); x2 -> qt quarters 2-3.
            nc.sync.dma_start(out=wk16t, in_=wk16[:, :, :])
            for kb in range(DB):
                nc.sync.dma_start(out=y16t[:, kb, 0:1024],
                                  in_=y16[:, kb, 0:1024])
            nc.sync.dma_start(out=wq16t, in_=wq16[:, :, :])
            for kb in range(DB):
                nc.sync.dma_start(out=x16t[:, kb, 0:1024],
                                  in_=x16[:, kb, 0:1024])
            nc.sync.dma_start(out=wv16t, in_=wv16[:, :, :])
            for kb in range(DB):
                nc.sync.dma_start(out=y16t[:, kb, 1024:2048],
                                  in_=y16[:, kb, 1024:2048])
            nc.sync.dma_start(out=wo16t, in_=wo16[:, :, :])
            for kb in range(DB):
                nc.sync.dma_start(out=x16t[:, kb, 1024:2048],
                                  in_=x16[:, kb, 1024:2048])

            for skb in range(SKB):
                nc.vector.memset(vp3[skb][:, :, Dh], 1.0)

            # ---- projection chunk emitters (share the S1 psum slot) ----
            def pj_tile(w):
                return ps1.tile([128, w], f32, name="s1", tag="s1",
                                padded_shape=[128, 1024])

            def emit_kt_chunk(ob, c4):
                ps = pj_tile(512)
                for kb in range(DB):
                    nc.tensor.matmul(
                        ps[:, :],
                        wk16t[:, kb, ob * 128:(ob + 1) * 128],
                        y16t[:, kb, c4 * 512:(c4 + 1) * 512],
                        start=(kb == 0), stop=(kb == DB - 1))
                nc.vector.tensor_copy(
                    kTp[ob][:, c4 * 512:(c4 + 1) * 512], ps[:, :])

            def emit_qt_chunk(ob, g2):
                ps = pj_tile(512)
                for kb in range(DB):
                    nc.tensor.matmul(
                        ps[:, :],
                        wq16t[:, kb, ob * 128:(ob + 1) * 128],
                        x16t[:, kb, g2 * 512:(g2 + 1) * 512],
                        start=(kb == 0), stop=(kb == DB - 1))
                nc.vector.tensor_copy(
                    qT[ob][:, g2 * 512:(g2 + 1) * 512], ps[:, :])

            def emit_vp_chunk(skb, hb):
                # v' for the two heads of pair hb over key block skb
                ps = pj_tile(128)
                for kb in range(DB):
                    nc.tensor.matmul(
                        ps[:, :],
                        y16t[:, kb, skb * 128:(skb + 1) * 128],
                        wv16t[:, kb, hb * 128:(hb + 1) * 128],
                        start=(kb == 0), stop=(kb == DB - 1))
                src = ps[:, :].rearrange("p (h c) -> p h c", c=Dh)
                nc.vector.tensor_copy(
                    vp3[skb][:, 2 * hb:2 * hb + 2, 0:Dh], src)

            def emit_o_chunk(g2, sqb):
                # partial output projection for 128 queries of quarter g2
                ps = pj_tile(D)
                q0 = sqb * 128
                for kb in range(OB):
                    for n0, n1 in ((0, 512), (512, D)):
                        nc.tensor.matmul(
                            ps[:, n0:n1],
                            vnorm[g2][kb][:, q0:q0 + 128],
                            wo16t[:, kb, n0:n1],
                            start=(kb == 0), stop=(kb == OB - 1))
                ot = o_pool.tile([128, D], f32, name="osb")
                nc.vector.tensor_copy(ot[:, :], ps[:, :])
                row0 = g2 * SQ4 + q0
                nc.sync.dma_start(out=out[row0:row0 + 128, :], in_=ot[:, :])

            # ---- task queue: (key, cost_cycles, emit_fn) in need order ----
            tasks = []

            def add_task(key, cost, fn):
                tasks.append((key, cost, fn))

            for hb in range(OB):
                u0 = hb * NQ  # first unit of this pair
                for g2 in range(NQ):
                    add_task(("qt", hb, g2), 3072,
                             (lambda ob=hb, g2=g2: emit_qt_chunk(ob, g2)))
                for c4 in range(4):
                    add_task(("kt", hb, c4), 3072,
                             (lambda ob=hb, c4=c4: emit_kt_chunk(ob, c4)))
                for s in range(SKB):
                    add_task(("vp", s, hb), 768,
                             (lambda s=s, hb=hb: emit_vp_chunk(s, hb)))

            state = {"budget": 0.0}

            def force(pred):
                rest = []
                for t in tasks:
                    if pred(t[0]):
                        t[2]()
                        state["budget"] -= t[1]
                    else:
                        rest.append(t)
                tasks[:] = rest

            def inject(budget_add):
                state["budget"] += budget_add
                while tasks and tasks[0][1] <= state["budget"]:
                    _, cost, fn = tasks.pop(0)
                    fn()
                    state["budget"] -= cost

            # ---- attention stream ----
            # bursts: (hb, g2, s): pair hb, query quarter g2, key block s.
            # hb outer so kt/vp forcing spreads across quarters; quarter
            # g2's o-proj launches after unit (hb=2, g2).
            units = [(hb, g2) for hb in range(OB) for g2 in range(NQ)]
            bursts = [(hb, g2, s) for (hb, g2) in units for s in range(SKB)]
            # slots alternate 2 bursts (S0, 4 banks) / 1 burst (S1)
            slots = []
            i = 0
            while i < len(bursts):
                n = 2 if len(slots) % 2 == 0 else 1
                slots.append(bursts[i:i + n])
                i += n

            vt_live = {}

            def emit_qk_slot(blist):
                n = len(blist)
                for (hb, g2, s) in blist:
                    if s == 0:
                        force(lambda t, hb=hb, g2=g2:
                              t[0] == "qt" and t[1] == hb and t[2] == g2)
                    if s % 4 == 0:
                        force(lambda t, hb=hb, c4=s // 4:
                              t[0] == "kt" and t[1] == hb and t[2] == c4)
                if n == 2:
                    st = ps0.tile([128, 2048], f32, name="s0", tag="s0")
                else:
                    st = pj_tile(1024)
                for i, (hb, g2, s) in enumerate(blist):
                    ks = slice(s * 128, (s + 1) * 128)
                    qs = slice(g2 * 512, (g2 + 1) * 512)
                    # two concurrent K=64 row-tiled matmuls: tile_position
                    # auto-derives (0,0) / (64,0) from the base partitions
                    nc.tensor.matmul(
                        st[:, i * 1024:i * 1024 + 512],
                        kTp[hb][0:64, ks], qT[hb][0:64, qs],
                        start=True, stop=True)
                    nc.tensor.matmul(
                        st[:, i * 1024 + 512:(i + 1) * 1024],
                        kTp[hb][64:128, ks], qT[hb][64:128, qs],
                        start=True, stop=True)
                pt = pt_pool.tile([128, n * 1024], bf16, name=f"pt{n}",
                                  tag=f"pt{n}")
                nc.scalar.activation(pt[:, :], st[:, :], EXP, scale=0.125)
                return pt

            def emit_vnorm(hb, g2, h_local):
                # h_local 0 (rows 0-63 of vnorm block) or 1 (rows 64-127)
                vt = vt_live.pop((hb, h_local))
                vals = nrm_pool.tile([64, SQ4], f32, name="vals")
                nc.vector.tensor_copy(vals[:, :], vt[0:64, :])
                den = nrm_pool.tile([1, SQ4], f32, name="den")
                nc.vector.tensor_copy(den[:, :], vt[64:65, :])
                rec = nrm_pool.tile([1, SQ4], f32, name="rec")
                # denominators positive, well inside normal fp32 range;
                # approx-fast needs a partition-0-aligned source
                nc.vector.reciprocal_approx_fast(rec[:, :], den[:, :])
                rbc = nrm_pool.tile([64, SQ4], f32, name="rbc")
                nc.gpsimd.partition_broadcast(rbc[:, :], rec[:, :])
                r0 = h_local * 64
                nc.vector.tensor_mul(
                    vnorm[g2][hb][r0:r0 + 64, :], vals[:, :], rbc[:, :])

            def emit_pv_burst(pt, i, hb, g2, s):
                for h_local, pool in ((0, vta_pool), (1, vtb_pool)):
                    if s == 0:
                        vt_live[(hb, h_local)] = pool.tile(
                            [65, SQ4], f32, name=f"vt{h_local}",
                            tag=f"vt{h_local}", padded_shape=[128, SQ4])
                    vt = vt_live[(hb, h_local)]
                    force(lambda t, s=s, hb=hb:
                          t[0] == "vp" and t[1] == s and t[2] == hb)
                    h = 2 * hb + h_local
                    nc.tensor.matmul(
                        vt[:, :],
                        vp16[s][:, h * 65:h * 65 + 65],
                        pt[:, i * 1024 + h_local * 512:
                           i * 1024 + (h_local + 1) * 512],
                        start=(s == 0), stop=(s == SKB - 1))
                    if s == SKB - 1:
                        emit_vnorm(hb, g2, h_local)
                if s == SKB - 1 and hb == OB - 1:
                    # quarter g2 complete: queue its output projection
                    for sqb in range(NQ):
                        add_task(("o", g2, sqb), 2304,
                                 (lambda g2=g2, sqb=sqb:
                                  emit_o_chunk(g2, sqb)))

            pending = []
            for blist in slots:
                pt = emit_qk_slot(blist)
                if pending:
                    ppt, pblist = pending.pop(0)
                    for i, (hb, g2, s) in enumerate(pblist):
                        emit_pv_burst(ppt, i, hb, g2, s)
                        inject(980)
                pending.append((pt, blist))
            ppt, pblist = pending.pop(0)
            for i, (hb, g2, s) in enumerate(pblist):
                emit_pv_burst(ppt, i, hb, g2, s)
                inject(980)

            force(lambda t: True)

    nc.compile()
    return nc


def _get_nc():
    if "nc" not in _cache:
        _cache["nc"] = _build_nc()
    return _cache["nc"]


def _host_fallback(x, y, mask, Wq, bq, Wkv, bkv, Wo, bo):
    Bb, Ss, _ = x.shape
    q = x @ Wq.T + bq
    kv = y @ Wkv.T + bkv
    q = q.reshape(Bb, Ss, H, Dh).transpose(0, 2, 1, 3)
    kv = kv.reshape(Bb, Ss, H, 2 * Dh).transpose(0, 2, 1, 3)
    k, v = kv[..., :Dh], kv[..., Dh:]
    scaled = np.einsum("bhqd,bhkd->bhqk", q, k) / np.sqrt(np.float32(Dh))
    scaled = scaled + mask
    scaled -= scaled.max(axis=-1, keepdims=True)
    e = np.exp(scaled)
    attn = e / e.sum(axis=-1, keepdims=True)
    values = np.einsum("bhqk,bhkd->bhqd", attn, v)
    values = values.transpose(0, 2, 1, 3).reshape(Bb, Ss, H * Dh)
    return (values @ Wo.T + bo).astype(np.float32)


def _blk(mat_t, dtype):
    """[768, N] row-blocked to [128, 6, N] in the given ml dtype."""
    n = mat_t.shape[1]
    return np.ascontiguousarray(
        mat_t.reshape(-1, 128, n).transpose(1, 0, 2)).astype(dtype)


def _run(inputs, trace=False, trace_cores=None):
    """Returns (full_output, BassKernelResults)."""
    import ml_dtypes
    from concourse.bass_utils import run_bass_kernel_spmd

    bf16 = ml_dtypes.bfloat16

    x = np.ascontiguousarray(np.asarray(inputs["x"], dtype=np.float32))
    y = np.ascontiguousarray(np.asarray(inputs["y"], dtype=np.float32))
    Wq = np.asarray(inputs["Wq"], dtype=np.float32)
    Wkv = np.asarray(inputs["Wkv"], dtype=np.float32)
    Wo = np.asarray(inputs["Wo"], dtype=np.float32)

    # Reference reshapes kv to [B,S,H,2*Dh]: per head, rows h*128..h*128+63 of
    # Wkv are the k-projection, rows h*128+64..h*128+127 the v-projection.
    in_maps = []
    for c in range(N_CORES):
        b, g = c // 2, c % 2
        heads = range(g * HC, (g + 1) * HC)
        k_rows = np.concatenate([np.arange(h * 128, h * 128 + Dh)
                                 for h in heads])
        v_rows = np.concatenate([np.arange(h * 128 + Dh, (h + 1) * 128)
                                 for h in heads])
        q_rows = np.concatenate([np.arange(h * Dh, (h + 1) * Dh)
                                 for h in heads])
        in_maps.append({
            "x16": _blk(x[b].T, bf16),
            "y16": _blk(y[b].T, bf16),
            "wq16": _blk(Wq[q_rows].T, bf16),
            "wk16": _blk(Wkv[k_rows].T, bf16),
            "wv16": _blk(Wkv[v_rows].T, bf16),
            "wo16": _blk(Wo[:, q_rows].T, bf16),
        })

    nc = _get_nc()
    res = run_bass_kernel_spmd(nc, in_maps, core_ids=list(range(N_CORES)),
                               trace=trace, trace_cores=trace_cores)
    out = np.empty((B, S, D), dtype=np.float32)
    for b in range(B):
        # host-side "all-reduce" of the two head-halves' partial projections
        out[b] = res.results[2 * b]["out"]
        out[b] += res.results[2 * b + 1]["out"]
    return out, res


def kernel(**inputs) -> np.ndarray:
    mask = np.asarray(inputs["mask"], dtype=np.float32)
    bq = np.asarray(inputs["bq"], dtype=np.float32)
    bkv = np.asarray(inputs["bkv"], dtype=np.float32)
    bo = np.asarray(inputs["bo"], dtype=np.float32)
    if mask.any() or bq.any() or bkv.any() or bo.any():
        # Device kernel hardcodes zero mask/biases; stay correct regardless.
        return _host_fallback(
            np.asarray(inputs["x"], dtype=np.float32),
            np.asarray(inputs["y"], dtype=np.float32),
            mask, np.asarray(inputs["Wq"], dtype=np.float32), bq,
            np.asarray(inputs["Wkv"], dtype=np.float32), bkv,
            np.asarray(inputs["Wo"], dtype=np.float32), bo)
    out, _ = _run(inputs)
    return out
